# revision 2
# baseline (speedup 1.0000x reference)
"""Trainium2 Bass kernel for nn_CNNCrossPatchBackbone (sparse cross-patch attention).

Strategy: 8 cores = 4 batches x {ctx self-attention, tgt cross-attention}.
The two MHAs of one batch share only the (read-only) context tokens, so the
work is fully task-parallel: no collectives. Each core runs an identical-shape
problem: 1024 q-tokens x 1024 kv-tokens, 16 heads of dim 64, D=1024, fp32.

Host side (sharding/index prep): stable argsort of is_context, token gather,
rope-cache gather by the clipped integer coords, a pair-split permutation of
the D axis (so rope pairs are contiguous row-tiles after the on-device
transpose), weight transposes/permutation, bias rows appended as a 9th
zero-padded 128-row tile, and the 1/sqrt(hd)=2^-3 score scale folded into the
q-projection weights (exact in fp32).

Device side per core:
  1. PE-transpose x (tok-major -> d-major), rope rotation on DVE.
  2. QKV projections: Q^T,K^T [dout, tok] for the score matmuls; V in natural
     [tok, dout] layout with a ones-column interleaved per head (softmax
     denominator comes out of the same PSUM accumulation as A@V).
  3. Per head: S^T = K_h^T^T Q_h^T (64-partition matmuls), exp on ACT
     (max-subtraction skipped: scores ~N(0,1)), O_aug = A^T^T V_aug
     accumulated over k in PSUM, per-partition reciprocal scale.
  4. PE-transpose O, output projection, DMA out.
"""

import sys

sys.path.insert(0, "/opt/trn_rl_repo")

import numpy as np

import concourse.bass as bass  # noqa: F401
import concourse.tile as tile
from concourse import bacc, mybir
from concourse.bass_utils import run_bass_kernel_spmd
from concourse.masks import make_identity

B, K, D, H = 4, 2048, 1024, 16
NCTX = K // 2
NTOK = 1024  # tokens per side after the ctx/tgt split
HD = D // H  # 64
IMAGE_SIZE = 224.0
MAX_POS = 1024
P = 128
DT = D // P  # 8 d-tiles
TT = NTOK // P  # 8 token-tiles
F32 = mybir.dt.float32

# pair-split permutation: [x-evens, x-odds, y-evens, y-odds]
PERM = np.concatenate(
    [
        np.arange(0, 512, 2),
        np.arange(1, 512, 2),
        np.arange(512, 1024, 2),
        np.arange(513, 1024, 2),
    ]
)


def build_nc():
    nc = bacc.Bacc("TRN2", target_bir_lowering=False, debug=False, num_devices=8)

    xq_ext = nc.dram_tensor("xq", [NTOK, D], F32, kind="ExternalInput")
    xkv_ext = nc.dram_tensor("xkv", [NTOK, D], F32, kind="ExternalInput")
    # [set(q,kv), {cx,sx,cy,sy}, jtile, p, tok]
    cs_ext = nc.dram_tensor("ropecs", [2, 4, 2, P, NTOK], F32, kind="ExternalInput")
    wqkv_ext = nc.dram_tensor("wqkvT", [DT + 1, P, 3 * D], F32, kind="ExternalInput")
    wo_ext = nc.dram_tensor("woT", [DT + 1, P, D], F32, kind="ExternalInput")
    out_ext = nc.dram_tensor("out", [NTOK, D], F32, kind="ExternalOutput")

    with tile.TileContext(nc) as tc:
        with tc.tile_pool(name="const", bufs=1) as cpool:
            ident = cpool.tile([P, P], F32)
            make_identity(nc, ident[:])
            ones_row = cpool.tile([P, NTOK], F32)
            nc.gpsimd.memset(ones_row[:], 0.0)
            nc.gpsimd.memset(ones_row[0:1, :], 1.0)

            with (
                tc.tile_pool(name="p_qt", bufs=TT) as p_qt,
                tc.tile_pool(name="p_kt", bufs=TT) as p_kt,
                tc.tile_pool(name="p_v", bufs=TT) as p_v,
            ):
                QT = [p_qt.tile([P, NTOK], F32, tag="qt", name=f"qt{i}") for i in range(DT)]
                KT = [p_kt.tile([P, NTOK], F32, tag="kt", name=f"kt{i}") for i in range(DT)]
                VA = [p_v.tile([P, H * (HD + 1)], F32, tag="va", name=f"va{i}") for i in range(TT)]

                # ---- phase 1+2 for the q set: build rope'd xqT, project Q ----
                _transpose_rope_project(
                    tc, nc, xq_ext, cs_ext, wqkv_ext, ident, ones_row, 0, QT, None, None
                )
                # ---- same for the kv set: project K and V ----
                _transpose_rope_project(
                    tc, nc, xkv_ext, cs_ext, wqkv_ext, ident, ones_row, 1, None, KT, VA
                )

                # ---- phase 3: attention ----
                with tc.tile_pool(name="p_o", bufs=TT) as p_o:
                    O = [p_o.tile([P, D], F32, tag="o", name=f"o{i}") for i in range(TT)]
                    with (
                        tc.tile_pool(name="p_a", bufs=18) as p_a,
                        tc.tile_pool(name="p_r", bufs=6) as p_r,
                        tc.tile_pool(name="ps_s", bufs=4, space="PSUM") as ps_s,
                        tc.tile_pool(name="ps_o", bufs=4, space="PSUM") as ps_o,
                    ):
                        for h in range(H):
                            qt = h // 2
                            po = (h % 2) * HD
                            a_tiles = []
                            for qh in range(2):
                                for kc in range(TT):
                                    s_ps = ps_s.tile([P, 512], F32, tag="s")
                                    nc.tensor.matmul(
                                        s_ps[:],
                                        KT[qt][po : po + HD, kc * P : (kc + 1) * P],
                                        QT[qt][po : po + HD, qh * 512 : (qh + 1) * 512],
                                        start=True,
                                        stop=True,
                                    )
                                    a_t = p_a.tile([P, 512], F32, tag="a")
                                    nc.scalar.activation(
                                        a_t[:], s_ps[:], mybir.ActivationFunctionType.Exp
                                    )
                                    a_tiles.append((qh, kc, a_t))
                            for qh in range(2):
                                for qc in range(4):
                                    o_ps = ps_o.tile([P, HD + 1], F32, tag="oacc")
                                    for kc in range(TT):
                                        a_t = a_tiles[qh * TT + kc][2]
                                        nc.tensor.matmul(
                                            o_ps[:],
                                            a_t[:, qc * P : (qc + 1) * P],
                                            VA[kc][:, h * (HD + 1) : (h + 1) * (HD + 1)],
                                            start=(kc == 0),
                                            stop=(kc == TT - 1),
                                        )
                                    qc8 = qh * 4 + qc
                                    r = p_r.tile([P, 1], F32, tag="r")
                                    nc.vector.reciprocal(r[:], o_ps[:, HD : HD + 1])
                                    nc.vector.tensor_scalar_mul(
                                        O[qc8][:, h * HD : (h + 1) * HD],
                                        o_ps[:, 0:HD],
                                        r[:],
                                    )

                    # ---- phase 4: output projection ----
                    with (
                        tc.tile_pool(name="p_wo", bufs=DT + 1) as p_wo,
                        tc.tile_pool(name="p_ot", bufs=10) as p_ot,
                        tc.tile_pool(name="p_y", bufs=3) as p_y,
                        tc.tile_pool(name="ps_t", bufs=2, space="PSUM") as ps_t,
                        tc.tile_pool(name="ps_y", bufs=4, space="PSUM") as ps_y,
                    ):
                        WO = []
                        for dt in range(DT + 1):
                            w_t = p_wo.tile([P, D], F32, tag="wo")
                            nc.sync.dma_start(w_t[:], wo_ext.ap()[dt])
                            WO.append(w_t)
                        for qc in range(TT):
                            ot_tiles = []
                            for dt in range(DT):
                                t_ps = ps_t.tile([P, P], F32, tag="t")
                                nc.tensor.transpose(
                                    t_ps[:], O[qc][:, dt * P : (dt + 1) * P], ident[:]
                                )
                                ot = p_ot.tile([P, P], F32, tag="ot")
                                nc.any.tensor_copy(ot[:], t_ps[:])
                                ot_tiles.append(ot)
                            y_t = p_y.tile([P, D], F32, tag="y")
                            for nh in range(2):
                                y_ps = ps_y.tile([P, 512], F32, tag="y")
                                for dt in range(DT + 1):
                                    lhsT = (
                                        ot_tiles[dt][:]
                                        if dt < DT
                                        else ones_row[:, qc * P : (qc + 1) * P]
                                    )
                                    nc.tensor.matmul(
                                        y_ps[:],
                                        lhsT,
                                        WO[dt][:, nh * 512 : (nh + 1) * 512],
                                        start=(dt == 0),
                                        stop=(dt == DT),
                                    )
                                nc.any.tensor_copy(y_t[:, nh * 512 : (nh + 1) * 512], y_ps[:])
                            nc.sync.dma_start(out_ext.ap()[qc * P : (qc + 1) * P, :], y_t[:])

    nc.compile()
    return nc


def _transpose_rope_project(tc, nc, x_ext, cs_ext, wqkv_ext, ident, ones_row, s, QT, KT, VA):
    """Build rope'd x^T (pair-split layout) for set s, then project.

    s=0 (q set): write Q^T tiles into QT.
    s=1 (kv set): write K^T tiles into KT and V (+ones cols) into VA.
    """
    with tc.tile_pool(name=f"p_xt{s}", bufs=DT) as p_xt:
        xT = [p_xt.tile([P, NTOK], F32, tag="xt", name=f"xt{s}_{i}") for i in range(DT)]

        with (
            tc.tile_pool(name=f"p_cs{s}", bufs=4) as p_cs,
            tc.tile_pool(name=f"p_tmp{s}", bufs=4) as p_tmp,
            tc.tile_pool(name=f"p_raw{s}", bufs=2) as p_raw,
            tc.tile_pool(name=f"ps_t{s}", bufs=4, space="PSUM") as ps_t,
        ):
            # transpose x into pair-split row tiles
            for tt in range(TT):
                raw = p_raw.tile([P, D], F32, tag="raw")
                nc.sync.dma_start(raw[:], x_ext.ap()[tt * P : (tt + 1) * P, :])
                for rt in range(DT):
                    t_ps = ps_t.tile([P, P], F32, tag="t")
                    nc.tensor.transpose(t_ps[:], raw[:, rt * P : (rt + 1) * P], ident[:])
                    nc.any.tensor_copy(xT[rt][:, tt * P : (tt + 1) * P], t_ps[:])

            # rope: groups (even_tile, odd_tile, cos_idx, sin_idx, jtile)
            for g in range(4):
                half = g // 2  # 0: x-half, 1: y-half
                j = g % 2  # jtile
                ev = xT[half * 4 + j]
                od = xT[half * 4 + 2 + j]
                c_t = p_cs.tile([P, NTOK], F32, tag="cs")
                nc.sync.dma_start(c_t[:], cs_ext.ap()[s, 2 * half + 0, j])
                s_t = p_cs.tile([P, NTOK], F32, tag="cs")
                nc.sync.dma_start(s_t[:], cs_ext.ap()[s, 2 * half + 1, j])
                t1 = p_tmp.tile([P, NTOK], F32, tag="tmp")
                t2 = p_tmp.tile([P, NTOK], F32, tag="tmp")
                t3 = p_tmp.tile([P, NTOK], F32, tag="tmp")
                t4 = p_tmp.tile([P, NTOK], F32, tag="tmp")
                nc.vector.tensor_mul(t1[:], ev[:], c_t[:])
                nc.vector.tensor_mul(t2[:], od[:], s_t[:])
                nc.vector.tensor_mul(t3[:], ev[:], s_t[:])
                nc.vector.tensor_mul(t4[:], od[:], c_t[:])
                nc.vector.tensor_sub(ev[:], t1[:], t2[:])
                nc.vector.tensor_add(od[:], t3[:], t4[:])

        # projections
        if s == 0:
            with (
                tc.tile_pool(name="p_wq", bufs=DT + 1) as p_w,
                tc.tile_pool(name="ps_p0", bufs=6, space="PSUM") as ps_p,
            ):
                W = _load_w(nc, p_w, wqkv_ext, 0)
                _proj_T(nc, ps_p, W, xT, ones_row, QT)
        else:
            with (
                tc.tile_pool(name="p_wk", bufs=DT + 1) as p_w,
                tc.tile_pool(name="ps_p1", bufs=6, space="PSUM") as ps_p,
            ):
                W = _load_w(nc, p_w, wqkv_ext, 1)
                _proj_T(nc, ps_p, W, xT, ones_row, KT)
            with (
                tc.tile_pool(name="p_wv", bufs=DT + 1) as p_w,
                tc.tile_pool(name="ps_p2", bufs=6, space="PSUM") as ps_p,
            ):
                W = _load_w(nc, p_w, wqkv_ext, 2)
                # V natural layout with interleaved ones columns
                for tt in range(TT):
                    va = VA[tt]
                    nc.gpsimd.memset(
                        va[:].rearrange("p (h c) -> p h c", c=HD + 1)[:, :, HD : HD + 1],
                        1.0,
                    )
                    for nh in range(2):
                        v_ps = ps_p.tile([P, 512], F32, tag="p")
                        for dt in range(DT + 1):
                            lhsT = (
                                xT[dt][:, tt * P : (tt + 1) * P]
                                if dt < DT
                                else ones_row[:, tt * P : (tt + 1) * P]
                            )
                            nc.tensor.matmul(
                                v_ps[:],
                                lhsT,
                                W[dt][:, nh * 512 : (nh + 1) * 512],
                                start=(dt == 0),
                                stop=(dt == DT),
                            )
                        out_ap = va[:].rearrange("p (h c) -> p h c", c=HD + 1)[
                            :, nh * 8 : (nh + 1) * 8, 0:HD
                        ]
                        nc.any.tensor_copy(
                            out_ap, v_ps[:].rearrange("p (h c) -> p h c", c=HD)
                        )


def _load_w(nc, pool, wqkv_ext, which):
    """Load one projection's weight tiles [P, D] (cols which*D..) x (DT+1)."""
    W = []
    for dt in range(DT + 1):
        w_t = pool.tile([P, D], F32, tag="w")
        nc.sync.dma_start(w_t[:], wqkv_ext.ap()[dt, :, which * D : (which + 1) * D])
        W.append(w_t)
    return W


def _proj_T(nc, ps_p, W, xT, ones_row, OUT):
    """OUT[c] = transposed projection [dout-chunk, tok]: lhsT=W slice, rhs=x^T."""
    for c in range(DT):
        out_t = OUT[c]
        for nh in range(2):
            ps = ps_p.tile([P, 512], F32, tag="p")
            for dt in range(DT + 1):
                rhs = xT[dt] if dt < DT else ones_row
                nc.tensor.matmul(
                    ps[:],
                    W[dt][:, c * P : (c + 1) * P],
                    rhs[:, nh * 512 : (nh + 1) * 512],
                    start=(dt == 0),
                    stop=(dt == DT),
                )
            nc.any.tensor_copy(out_t[:, nh * 512 : (nh + 1) * 512], ps[:])


# ---------------------------------------------------------------------------
# host side
# ---------------------------------------------------------------------------

def host_prep(x, coords, is_context, rope_cache,
              ctx_in_w, ctx_in_b, ctx_out_w, ctx_out_b,
              tgt_in_w, tgt_in_b, tgt_out_w, tgt_out_b):
    """Compute per-core input maps + the scatter indices."""
    x = np.asarray(x, np.float32)
    coords = np.asarray(coords, np.float32)
    is_context = np.asarray(is_context, bool)
    rope_cache = np.asarray(rope_cache, np.float32)

    keys = np.where(is_context, 0, 1).astype(np.int32)
    order = np.argsort(keys, axis=1, kind="stable")
    ctx_idx = order[:, :NCTX]
    tgt_idx = order[:, NCTX:]

    # rope positions (mirror reference fp32 arithmetic)
    cn = np.clip(
        coords / np.float32(IMAGE_SIZE) * np.float32(MAX_POS - 1), 0, MAX_POS - 1
    )
    y_pos = cn[..., 0].astype(np.int32)
    x_pos = cn[..., 1].astype(np.int32)
    # [B, K, 256] each
    cx_all = rope_cache[x_pos, :, 0]
    sx_all = rope_cache[x_pos, :, 1]
    cy_all = rope_cache[y_pos, :, 0]
    sy_all = rope_cache[y_pos, :, 1]

    def w_pack(in_w, in_b, out_w, out_b):
        w = np.array(in_w, np.float32)
        bvec = np.array(in_b, np.float32)
        w[0:D] *= np.float32(0.125)
        bvec = bvec.copy()
        bvec[0:D] *= np.float32(0.125)
        wT = np.ascontiguousarray(w.T)[PERM]
        wqkvT = np.concatenate(
            [wT, bvec[None, :], np.zeros((P - 1, 3 * D), np.float32)]
        ).reshape(DT + 1, P, 3 * D)
        woT = np.concatenate(
            [
                np.ascontiguousarray(np.asarray(out_w, np.float32).T),
                np.asarray(out_b, np.float32)[None, :],
                np.zeros((P - 1, D), np.float32),
            ]
        ).reshape(DT + 1, P, D)
        return np.ascontiguousarray(wqkvT), np.ascontiguousarray(woT)

    packs = [w_pack(ctx_in_w, ctx_in_b, ctx_out_w, ctx_out_b),
             w_pack(tgt_in_w, tgt_in_b, tgt_out_w, tgt_out_b)]

    def cs_pack(b, idx):
        # [4, 2, P, NTOK]
        out = np.empty((4, 2, P, NTOK), np.float32)
        for i, arr in enumerate((cx_all, sx_all, cy_all, sy_all)):
            t = arr[b][idx].T  # [256, NTOK]
            out[i] = t.reshape(2, P, NTOK)
        return out

    in_maps = []
    scatter = []
    for c in range(8):
        b, role = c // 2, c % 2
        q_idx = ctx_idx[b] if role == 0 else tgt_idx[b]
        kv_idx = ctx_idx[b]
        ropecs = np.stack([cs_pack(b, q_idx), cs_pack(b, kv_idx)])
        wqkvT, woT = packs[role]
        in_maps.append({
            "xq": np.ascontiguousarray(x[b][q_idx][:, PERM]),
            "xkv": np.ascontiguousarray(x[b][kv_idx][:, PERM]),
            "ropecs": np.ascontiguousarray(ropecs),
            "wqkvT": wqkvT,
            "woT": woT,
        })
        scatter.append((b, q_idx))
    return in_maps, scatter


_NC_CACHE = None


def kernel(**inputs):
    global _NC_CACHE
    in_maps, scatter = host_prep(**inputs)
    if _NC_CACHE is None:
        _NC_CACHE = build_nc()
    nc = _NC_CACHE
    res = run_bass_kernel_spmd(nc, in_maps, core_ids=list(range(8)))
    x = np.asarray(inputs["x"], np.float32)
    out = np.zeros_like(x)
    for c in range(8):
        b, q_idx = scatter[c]
        out[b][q_idx] = res.results[c]["out"]
    return out


# revision 10
# speedup vs baseline: 1.5979x; 1.5979x over previous
"""Trainium2 Bass kernel for nn_CNNCrossPatchBackbone (sparse cross-patch attention).

Strategy: 8 cores = 4 batches x {ctx self-attention, tgt cross-attention}.
The two MHAs of one batch share only the (read-only) context tokens, so the
work is fully task-parallel: no collectives. Each core runs an identical-shape
problem: 1024 q-tokens x 1024 kv-tokens, 16 heads of dim 64, D=1024.

Matmuls run in float32r (single-pass fp32, ~1 cycle/row for >=256-wide moving
operand, ~1e-4 matmul rel err vs 4 cycles/row for exact fp32). Everything else
(rope, exp, scaling) is fp32.

Host side (sharding/index prep): stable argsort of is_context, token gather,
rope-cache gather by the clipped integer coords, a pair-split permutation of
the D axis (so rope pairs are contiguous row-tiles after the on-device
transpose), weight transposes/permutation, bias rows appended as a 9th
zero-padded 128-row tile, and the 1/sqrt(hd)=2^-3 score scale folded into the
q-projection weights (exact in fp32).

Device side per core:
  1. PE-transpose x (tok-major -> d-major), rope rotation on DVE.
  2. QKV projections: Q^T,K^T [dout, tok] for the score matmuls; V in natural
     [tok, dout] layout with a ones-column interleaved per head.
  3. Per head: S^T = K_h^T^T Q_h^T (64-partition matmuls), exp on ACT
     (max-subtraction skipped: scores ~N(0,1)), O^T_aug = V_aug^T A^T
     accumulated over k in PSUM; row 64 is the softmax denominator, which is
     reciprocal'd, partition-broadcast, and multiplied in on DVE. O^T lands
     directly in the [d, tok] layout the output projection consumes.
  4. Output projection, DMA out.
"""

import sys

sys.path.insert(0, "/opt/trn_rl_repo")

import numpy as np

import concourse.bass as bass  # noqa: F401
import concourse.tile as tile
from concourse import bacc, mybir
from concourse.bass_utils import run_bass_kernel_spmd
from concourse.masks import make_identity

B, K, D, H = 4, 2048, 1024, 16
NCTX = K // 2
NTOK = 1024  # tokens per side after the ctx/tgt split
HD = D // H  # 64
IMAGE_SIZE = 224.0
MAX_POS = 1024
P = 128
DT = D // P  # 8 d-tiles
TT = NTOK // P  # 8 token-tiles
F32 = mybir.dt.float32
F32R = mybir.dt.float32r

# pair-split permutation: [x-evens, x-odds, y-evens, y-odds]
PERM = np.concatenate(
    [
        np.arange(0, 512, 2),
        np.arange(1, 512, 2),
        np.arange(512, 1024, 2),
        np.arange(513, 1024, 2),
    ]
)


def build_nc():
    nc = bacc.Bacc("TRN2", target_bir_lowering=False, debug=False, num_devices=8)

    xq_ext = nc.dram_tensor("xq", [NTOK, D], F32, kind="ExternalInput")
    xkv_ext = nc.dram_tensor("xkv", [NTOK, D], F32, kind="ExternalInput")
    # [set(q,kv), {cx,sx,cy,sy}, jtile, p, tok]
    cs_ext = nc.dram_tensor("ropecs", [2, 4, 2, P, NTOK], F32, kind="ExternalInput")
    wqkv_ext = nc.dram_tensor("wqkvT", [DT + 1, P, 3 * D], F32, kind="ExternalInput")
    wo_ext = nc.dram_tensor("woT", [DT + 1, P, D], F32, kind="ExternalInput")
    out_ext = nc.dram_tensor("out", [NTOK, D], F32, kind="ExternalOutput")

    with tile.TileContext(nc) as tc:
        with tc.tile_pool(name="const", bufs=1) as cpool:
            ident = cpool.tile([P, P], F32)
            make_identity(nc, ident[:])
            ones_f32 = cpool.tile([P, NTOK], F32)
            nc.gpsimd.memset(ones_f32[:], 0.0)
            nc.gpsimd.memset(ones_f32[0:1, :], 1.0)
            ones_row = cpool.tile([P, NTOK], F32R)
            nc.vector.tensor_copy(ones_row[:], ones_f32[:])
            all1 = cpool.tile([P, H], F32)
            nc.gpsimd.memset(all1[:], 1.0)

            with (
                tc.tile_pool(name="p_qt", bufs=TT) as p_qt,
                tc.tile_pool(name="p_kt", bufs=TT) as p_kt,
                tc.tile_pool(name="p_v", bufs=TT) as p_v,
            ):
                QT = [p_qt.tile([P, NTOK], F32R, tag="qt", name=f"qt{i}") for i in range(DT)]
                KT = [p_kt.tile([P, NTOK], F32R, tag="kt", name=f"kt{i}") for i in range(DT)]
                VA = [p_v.tile([P, H * (HD + 1)], F32R, tag="va", name=f"va{i}") for i in range(TT)]

                # ---- phase 1+2 for the q set: build rope'd xqT, project Q ----
                _transpose_rope_project(
                    tc, nc, xq_ext, cs_ext, wqkv_ext, ident, ones_row, all1, 0, QT, None, None
                )
                # ---- same for the kv set: project K and V ----
                _transpose_rope_project(
                    tc, nc, xkv_ext, cs_ext, wqkv_ext, ident, ones_row, all1, 1, None, KT, VA
                )

                # ---- phase 3: attention ----
                with tc.tile_pool(name="p_ot", bufs=DT) as p_ot:
                    OT = [p_ot.tile([P, NTOK], F32R, tag="ot", name=f"ot{i}") for i in range(DT)]
                    with (
                        tc.tile_pool(name="p_a", bufs=16) as p_a,
                        tc.tile_pool(name="p_r", bufs=2) as p_r,
                        tc.tile_pool(name="p_rb", bufs=3) as p_rb,
                        tc.tile_pool(name="ps_s", bufs=3, space="PSUM") as ps_s,
                        tc.tile_pool(name="ps_o", bufs=3, space="PSUM") as ps_o,
                        tc.tile_pool(name="ps_rb", bufs=2, space="PSUM") as ps_rb,
                    ):
                        for h in range(H):
                            qt = h // 2
                            po = (h % 2) * HD
                            a_tiles = []
                            for qh in range(2):
                                for kc in range(TT):
                                    s_ps = ps_s.tile([P, 512], F32, tag="s")
                                    nc.tensor.matmul(
                                        s_ps[:],
                                        KT[qt][po : po + HD, kc * P : (kc + 1) * P],
                                        QT[qt][po : po + HD, qh * 512 : (qh + 1) * 512],
                                        start=True,
                                        stop=True,
                                    )
                                    a_t = p_a.tile([P, 512], F32R, tag="a")
                                    nc.scalar.activation(
                                        a_t[:], s_ps[:], mybir.ActivationFunctionType.Exp
                                    )
                                    a_tiles.append(a_t)
                            for qh in range(2):
                                o_ps = ps_o.tile([HD + 1, 512], F32, tag="oacc")
                                for kc in range(TT):
                                    nc.tensor.matmul(
                                        o_ps[:],
                                        VA[kc][:, h * (HD + 1) : (h + 1) * (HD + 1)],
                                        a_tiles[qh * TT + kc][:],
                                        start=(kc == 0),
                                        stop=(kc == TT - 1),
                                    )
                                qs = slice(qh * 512, (qh + 1) * 512)
                                r1 = p_r.tile([1, 512], F32R, tag="r")
                                with nc.allow_low_precision(reason="f32r recip for PE broadcast"):
                                    nc.vector.reciprocal(r1[:], o_ps[HD : HD + 1, :])
                                # broadcast r1 across 64 partitions via a K=1 matmul
                                rb_ps = ps_rb.tile([HD, 512], F32, tag="rbps")
                                nc.tensor.matmul(
                                    rb_ps[:],
                                    ones_row[0:1, 0:HD],
                                    r1[:],
                                    start=True,
                                    stop=True,
                                )
                                rb = p_rb.tile([HD, 512], F32, tag="rb")
                                nc.any.tensor_copy(rb[:], rb_ps[:])
                                nc.vector.tensor_mul(
                                    OT[qt][po : po + HD, qs], o_ps[0:HD, :], rb[:]
                                )

                    # ---- phase 4: output projection ----
                    with (
                        tc.tile_pool(name="p_wo", bufs=DT + 1) as p_wo,
                        tc.tile_pool(name="p_wraw", bufs=3) as p_wraw,
                        tc.tile_pool(name="p_y", bufs=3) as p_y,
                        tc.tile_pool(name="ps_y", bufs=4, space="PSUM") as ps_y,
                    ):
                        WO = _load_w(nc, p_wo, p_wraw, wo_ext, None)
                        for qc in range(TT):
                            y_t = p_y.tile([P, D], F32, tag="y")
                            for nh in range(2):
                                y_ps = ps_y.tile([P, 512], F32, tag="y")
                                for dt in range(DT + 1):
                                    lhsT = (
                                        OT[dt][:, qc * P : (qc + 1) * P]
                                        if dt < DT
                                        else ones_row[:, qc * P : (qc + 1) * P]
                                    )
                                    nc.tensor.matmul(
                                        y_ps[:],
                                        lhsT,
                                        WO[dt][:, nh * 512 : (nh + 1) * 512],
                                        start=(dt == 0),
                                        stop=(dt == DT),
                                    )
                                nc.any.tensor_copy(y_t[:, nh * 512 : (nh + 1) * 512], y_ps[:])
                            nc.sync.dma_start(out_ext.ap()[qc * P : (qc + 1) * P, :], y_t[:])

    nc.compile()
    return nc


def _transpose_rope_project(tc, nc, x_ext, cs_ext, wqkv_ext, ident, ones_row, all1, s, QT, KT, VA):
    """Build rope'd x^T (pair-split layout, float32r) for set s, then project.

    s=0 (q set): write Q^T tiles into QT.
    s=1 (kv set): write K^T tiles into KT and V (+ones cols) into VA.
    """
    with tc.tile_pool(name=f"p_xt{s}", bufs=DT) as p_xt:
        xT = [p_xt.tile([P, NTOK], F32R, tag="xt", name=f"xt{s}_{i}") for i in range(DT)]

        with (
            tc.tile_pool(name=f"p_cs{s}", bufs=4) as p_cs,
            tc.tile_pool(name=f"p_tmp{s}", bufs=4) as p_tmp,
            tc.tile_pool(name=f"p_raw{s}", bufs=2) as p_raw,
            tc.tile_pool(name=f"ps_t{s}", bufs=4, space="PSUM") as ps_t,
        ):
            # transpose x into pair-split row tiles
            for tt in range(TT):
                raw = p_raw.tile([P, D], F32, tag="raw")
                nc.sync.dma_start(raw[:], x_ext.ap()[tt * P : (tt + 1) * P, :])
                for rt in range(DT):
                    t_ps = ps_t.tile([P, P], F32, tag="t")
                    nc.tensor.transpose(t_ps[:], raw[:, rt * P : (rt + 1) * P], ident[:])
                    nc.any.tensor_copy(xT[rt][:, tt * P : (tt + 1) * P], t_ps[:])

            # rope: groups (even_tile, odd_tile, cos_idx, sin_idx, jtile)
            for g in range(4):
                half = g // 2  # 0: x-half, 1: y-half
                j = g % 2  # jtile
                ev = xT[half * 4 + j]
                od = xT[half * 4 + 2 + j]
                c_t = p_cs.tile([P, NTOK], F32, tag="cs")
                nc.sync.dma_start(c_t[:], cs_ext.ap()[s, 2 * half + 0, j])
                s_t = p_cs.tile([P, NTOK], F32, tag="cs")
                nc.sync.dma_start(s_t[:], cs_ext.ap()[s, 2 * half + 1, j])
                t1 = p_tmp.tile([P, NTOK], F32, tag="tmp")
                t2 = p_tmp.tile([P, NTOK], F32, tag="tmp")
                t3 = p_tmp.tile([P, NTOK], F32, tag="tmp")
                t4 = p_tmp.tile([P, NTOK], F32, tag="tmp")
                nc.vector.tensor_mul(t1[:], ev[:], c_t[:])
                nc.vector.tensor_mul(t2[:], od[:], s_t[:])
                nc.vector.tensor_mul(t3[:], ev[:], s_t[:])
                nc.vector.tensor_mul(t4[:], od[:], c_t[:])
                nc.vector.tensor_sub(ev[:], t1[:], t2[:])
                nc.vector.tensor_add(od[:], t3[:], t4[:])

        # projections
        if s == 0:
            with (
                tc.tile_pool(name="p_wq", bufs=DT + 1) as p_w,
                tc.tile_pool(name="p_wraw0", bufs=3) as p_wraw,
                tc.tile_pool(name="ps_p0", bufs=6, space="PSUM") as ps_p,
            ):
                W = _load_w(nc, p_w, p_wraw, wqkv_ext, 0)
                _proj_T(nc, ps_p, W, xT, ones_row, QT)
        else:
            with (
                tc.tile_pool(name="p_wk", bufs=DT + 1) as p_w,
                tc.tile_pool(name="p_wraw1", bufs=3) as p_wraw,
                tc.tile_pool(name="ps_p1", bufs=6, space="PSUM") as ps_p,
            ):
                W = _load_w(nc, p_w, p_wraw, wqkv_ext, 1)
                _proj_T(nc, ps_p, W, xT, ones_row, KT)
            with (
                tc.tile_pool(name="p_wv", bufs=DT + 1) as p_w,
                tc.tile_pool(name="p_wraw2", bufs=3) as p_wraw,
                tc.tile_pool(name="ps_p2", bufs=6, space="PSUM") as ps_p,
            ):
                W = _load_w(nc, p_w, p_wraw, wqkv_ext, 2)
                # V natural layout with interleaved ones columns
                for tt in range(TT):
                    va = VA[tt]
                    nc.vector.tensor_copy(
                        va[:].rearrange("p (h c) -> p h c", c=HD + 1)[:, :, HD : HD + 1],
                        all1[:].rearrange("p (h c) -> p h c", c=1),
                    )
                    for nh in range(2):
                        v_ps = ps_p.tile([P, 512], F32, tag="p")
                        for dt in range(DT + 1):
                            lhsT = (
                                xT[dt][:, tt * P : (tt + 1) * P]
                                if dt < DT
                                else ones_row[:, tt * P : (tt + 1) * P]
                            )
                            nc.tensor.matmul(
                                v_ps[:],
                                lhsT,
                                W[dt][:, nh * 512 : (nh + 1) * 512],
                                start=(dt == 0),
                                stop=(dt == DT),
                            )
                        out_ap = va[:].rearrange("p (h c) -> p h c", c=HD + 1)[
                            :, nh * 8 : (nh + 1) * 8, 0:HD
                        ]
                        nc.any.tensor_copy(
                            out_ap, v_ps[:].rearrange("p (h c) -> p h c", c=HD)
                        )


def _load_w(nc, pool, rawpool, w_ext, which):
    """DMA one projection's weight tiles and cast-copy them to float32r."""
    W = []
    for dt in range(DT + 1):
        raw = rawpool.tile([P, D], F32, tag="wraw", name=f"wraw{dt}")
        if which is None:
            nc.sync.dma_start(raw[:], w_ext.ap()[dt])
        else:
            nc.sync.dma_start(raw[:], w_ext.ap()[dt, :, which * D : (which + 1) * D])
        w_t = pool.tile([P, D], F32R, tag="w", name=f"w{dt}")
        nc.any.tensor_copy(w_t[:], raw[:])
        W.append(w_t)
    return W


def _proj_T(nc, ps_p, W, xT, ones_row, OUT):
    """OUT[c] = transposed projection [dout-chunk, tok]: lhsT=W slice, rhs=x^T."""
    for c in range(DT):
        out_t = OUT[c]
        for nh in range(2):
            ps = ps_p.tile([P, 512], F32, tag="p")
            for dt in range(DT + 1):
                rhs = xT[dt] if dt < DT else ones_row
                nc.tensor.matmul(
                    ps[:],
                    W[dt][:, c * P : (c + 1) * P],
                    rhs[:, nh * 512 : (nh + 1) * 512],
                    start=(dt == 0),
                    stop=(dt == DT),
                )
            nc.any.tensor_copy(out_t[:, nh * 512 : (nh + 1) * 512], ps[:])


# ---------------------------------------------------------------------------
# host side
# ---------------------------------------------------------------------------

def host_prep(x, coords, is_context, rope_cache,
              ctx_in_w, ctx_in_b, ctx_out_w, ctx_out_b,
              tgt_in_w, tgt_in_b, tgt_out_w, tgt_out_b):
    """Compute per-core input maps + the scatter indices."""
    x = np.asarray(x, np.float32)
    coords = np.asarray(coords, np.float32)
    is_context = np.asarray(is_context, bool)
    rope_cache = np.asarray(rope_cache, np.float32)

    keys = np.where(is_context, 0, 1).astype(np.int32)
    order = np.argsort(keys, axis=1, kind="stable")
    ctx_idx = order[:, :NCTX]
    tgt_idx = order[:, NCTX:]

    # rope positions (mirror reference fp32 arithmetic)
    cn = np.clip(
        coords / np.float32(IMAGE_SIZE) * np.float32(MAX_POS - 1), 0, MAX_POS - 1
    )
    y_pos = cn[..., 0].astype(np.int32)
    x_pos = cn[..., 1].astype(np.int32)
    # [B, K, 256] each
    cx_all = rope_cache[x_pos, :, 0]
    sx_all = rope_cache[x_pos, :, 1]
    cy_all = rope_cache[y_pos, :, 0]
    sy_all = rope_cache[y_pos, :, 1]

    def w_pack(in_w, in_b, out_w, out_b):
        w = np.array(in_w, np.float32)
        bvec = np.array(in_b, np.float32)
        w[0:D] *= np.float32(0.125)
        bvec = bvec.copy()
        bvec[0:D] *= np.float32(0.125)
        wT = np.ascontiguousarray(w.T)[PERM]
        wqkvT = np.concatenate(
            [wT, bvec[None, :], np.zeros((P - 1, 3 * D), np.float32)]
        ).reshape(DT + 1, P, 3 * D)
        woT = np.concatenate(
            [
                np.ascontiguousarray(np.asarray(out_w, np.float32).T),
                np.asarray(out_b, np.float32)[None, :],
                np.zeros((P - 1, D), np.float32),
            ]
        ).reshape(DT + 1, P, D)
        return np.ascontiguousarray(wqkvT), np.ascontiguousarray(woT)

    packs = [w_pack(ctx_in_w, ctx_in_b, ctx_out_w, ctx_out_b),
             w_pack(tgt_in_w, tgt_in_b, tgt_out_w, tgt_out_b)]

    def cs_pack(b, idx):
        # [4, 2, P, NTOK]
        out = np.empty((4, 2, P, NTOK), np.float32)
        for i, arr in enumerate((cx_all, sx_all, cy_all, sy_all)):
            t = arr[b][idx].T  # [256, NTOK]
            out[i] = t.reshape(2, P, NTOK)
        return out

    in_maps = []
    scatter = []
    for c in range(8):
        b, role = c // 2, c % 2
        q_idx = ctx_idx[b] if role == 0 else tgt_idx[b]
        kv_idx = ctx_idx[b]
        ropecs = np.stack([cs_pack(b, q_idx), cs_pack(b, kv_idx)])
        wqkvT, woT = packs[role]
        in_maps.append({
            "xq": np.ascontiguousarray(x[b][q_idx][:, PERM]),
            "xkv": np.ascontiguousarray(x[b][kv_idx][:, PERM]),
            "ropecs": np.ascontiguousarray(ropecs),
            "wqkvT": wqkvT,
            "woT": woT,
        })
        scatter.append((b, q_idx))
    return in_maps, scatter


_NC_CACHE = None


def kernel(**inputs):
    global _NC_CACHE
    in_maps, scatter = host_prep(**inputs)
    if _NC_CACHE is None:
        _NC_CACHE = build_nc()
    nc = _NC_CACHE
    res = run_bass_kernel_spmd(nc, in_maps, core_ids=list(range(8)))
    x = np.asarray(inputs["x"], np.float32)
    out = np.zeros_like(x)
    for c in range(8):
        b, q_idx = scatter[c]
        out[b][q_idx] = res.results[c]["out"]
    return out


# revision 13
# speedup vs baseline: 1.6983x; 1.0628x over previous
"""Trainium2 Bass kernel for nn_CNNCrossPatchBackbone (sparse cross-patch attention).

Strategy: 8 cores = 4 batches x {ctx self-attention, tgt cross-attention}.
The two MHAs of one batch share only the (read-only) context tokens, so the
work is fully task-parallel: no collectives. Each core runs an identical-shape
problem: 1024 q-tokens x 1024 kv-tokens, 16 heads of dim 64, D=1024.

Matmuls run in float32r (single-pass fp32, ~1 cycle/row for >=256-wide moving
operand, ~1e-4 matmul rel err vs 4 cycles/row for exact fp32). Everything else
(rope, exp, scaling) is fp32.

Host side (sharding/index prep): stable argsort of is_context, token gather,
rope-cache gather by the clipped integer coords, a pair-split permutation of
the D axis (so rope pairs are contiguous row-tiles after the on-device
transpose), weight transposes/permutation, bias rows appended as a 9th
zero-padded 128-row tile, and the 1/sqrt(hd)=2^-3 score scale folded into the
q-projection weights (exact in fp32).

Device side per core:
  1. PE-transpose x (tok-major -> d-major), rope rotation on DVE.
  2. QKV projections: Q^T,K^T [dout, tok] for the score matmuls; V in natural
     [tok, dout] layout with a ones-column interleaved per head.
  3. Per head: S^T = K_h^T^T Q_h^T (64-partition matmuls), exp on ACT
     (max-subtraction skipped: scores ~N(0,1)), O^T_aug = V_aug^T A^T
     accumulated over k in PSUM; row 64 is the softmax denominator, which is
     reciprocal'd, partition-broadcast, and multiplied in on DVE. O^T lands
     directly in the [d, tok] layout the output projection consumes.
  4. Output projection, DMA out.
"""

import sys

sys.path.insert(0, "/opt/trn_rl_repo")

import numpy as np

import concourse.bass as bass  # noqa: F401
import concourse.tile as tile
from concourse import bacc, mybir
from concourse.bass_utils import run_bass_kernel_spmd
from concourse.masks import make_identity

B, K, D, H = 4, 2048, 1024, 16
NCTX = K // 2
NTOK = 1024  # tokens per side after the ctx/tgt split
HD = D // H  # 64
IMAGE_SIZE = 224.0
MAX_POS = 1024
P = 128
DT = D // P  # 8 d-tiles
TT = NTOK // P  # 8 token-tiles
F32 = mybir.dt.float32
F32R = mybir.dt.float32r

# pair-split permutation: [x-evens, x-odds, y-evens, y-odds]
PERM = np.concatenate(
    [
        np.arange(0, 512, 2),
        np.arange(1, 512, 2),
        np.arange(512, 1024, 2),
        np.arange(513, 1024, 2),
    ]
)


def build_nc():
    nc = bacc.Bacc("TRN2", target_bir_lowering=False, debug=False, num_devices=8)

    xq_ext = nc.dram_tensor("xq", [NTOK, D], F32, kind="ExternalInput")
    xkv_ext = nc.dram_tensor("xkv", [NTOK, D], F32, kind="ExternalInput")
    # [set(q,kv), {cx,sx,cy,sy}, jtile, p, tok]
    cs_ext = nc.dram_tensor("ropecs", [2, 4, 2, P, NTOK], F32, kind="ExternalInput")
    wqkv_ext = nc.dram_tensor("wqkvT", [DT + 1, P, 3 * D], F32, kind="ExternalInput")
    wo_ext = nc.dram_tensor("woT", [DT + 1, P, D], F32, kind="ExternalInput")
    out_ext = nc.dram_tensor("out", [NTOK, D], F32, kind="ExternalOutput")

    with tile.TileContext(nc) as tc:
        with tc.tile_pool(name="const", bufs=1) as cpool:
            ident = cpool.tile([P, P], F32)
            make_identity(nc, ident[:])
            ones_f32 = cpool.tile([P, NTOK], F32)
            nc.gpsimd.memset(ones_f32[:], 0.0)
            nc.gpsimd.memset(ones_f32[0:1, :], 1.0)
            ones_row = cpool.tile([P, NTOK], F32R)
            nc.vector.tensor_copy(ones_row[:], ones_f32[:])
            all1 = cpool.tile([P, H], F32)
            nc.gpsimd.memset(all1[:], 1.0)

            with (
                tc.tile_pool(name="p_qt", bufs=TT) as p_qt,
                tc.tile_pool(name="p_kt", bufs=TT) as p_kt,
                tc.tile_pool(name="p_v", bufs=TT) as p_v,
            ):
                QT = [p_qt.tile([P, NTOK], F32R, tag="qt", name=f"qt{i}") for i in range(DT)]
                KT = [p_kt.tile([P, NTOK], F32R, tag="kt", name=f"kt{i}") for i in range(DT)]
                VA = [p_v.tile([P, H * (HD + 1)], F32R, tag="va", name=f"va{i}") for i in range(TT)]

                # ---- phase 1+2 for the q set: build rope'd xqT, project Q ----
                _transpose_rope_project(
                    tc, nc, xq_ext, cs_ext, wqkv_ext, ident, ones_row, all1, 0, QT, None, None
                )
                # ---- same for the kv set: project K and V ----
                _transpose_rope_project(
                    tc, nc, xkv_ext, cs_ext, wqkv_ext, ident, ones_row, all1, 1, None, KT, VA
                )

                # ---- phase 3: attention ----
                with tc.tile_pool(name="p_ot", bufs=DT) as p_ot:
                    OT = [p_ot.tile([P, NTOK], F32R, tag="ot", name=f"ot{i}") for i in range(DT)]
                    with (
                        tc.tile_pool(name="p_a", bufs=20) as p_a,
                        tc.tile_pool(name="p_r", bufs=2) as p_r,
                        tc.tile_pool(name="p_rb", bufs=3) as p_rb,
                        tc.tile_pool(name="ps_s", bufs=4, space="PSUM") as ps_s,
                        tc.tile_pool(name="ps_o", bufs=3, space="PSUM") as ps_o,
                        tc.tile_pool(name="ps_rb", bufs=1, space="PSUM") as ps_rb,
                    ):
                        for h in range(H):
                            qt = h // 2
                            po = (h % 2) * HD
                            a_tiles = []
                            for qh in range(2):
                                for kc in range(TT):
                                    s_ps = ps_s.tile([P, 512], F32, tag="s")
                                    nc.tensor.matmul(
                                        s_ps[:],
                                        KT[qt][po : po + HD, kc * P : (kc + 1) * P],
                                        QT[qt][po : po + HD, qh * 512 : (qh + 1) * 512],
                                        start=True,
                                        stop=True,
                                    )
                                    a_t = p_a.tile([P, 512], F32R, tag="a")
                                    nc.scalar.activation(
                                        a_t[:], s_ps[:], mybir.ActivationFunctionType.Exp
                                    )
                                    a_tiles.append(a_t)
                            for qh in range(2):
                                o_ps = ps_o.tile([HD + 1, 512], F32, tag="oacc")
                                for kc in range(TT):
                                    nc.tensor.matmul(
                                        o_ps[:],
                                        VA[kc][:, h * (HD + 1) : (h + 1) * (HD + 1)],
                                        a_tiles[qh * TT + kc][:],
                                        start=(kc == 0),
                                        stop=(kc == TT - 1),
                                    )
                                qs = slice(qh * 512, (qh + 1) * 512)
                                dn = p_r.tile([1, 512], F32, tag="dn")
                                nc.vector.tensor_copy(dn[:], o_ps[HD : HD + 1, :])
                                r1 = p_r.tile([1, 512], F32, tag="r")
                                nc.vector.reciprocal_approx_fast(r1[:], dn[:])
                                r1r = p_r.tile([1, 512], F32R, tag="r1r")
                                nc.vector.tensor_copy(r1r[:], r1[:])
                                # broadcast r1 across 64 partitions via a K=1 matmul
                                rb_ps = ps_rb.tile([HD, 512], F32, tag="rbps")
                                nc.tensor.matmul(
                                    rb_ps[:],
                                    ones_row[0:1, 0:HD],
                                    r1r[:],
                                    start=True,
                                    stop=True,
                                )
                                rb = p_rb.tile([HD, 512], F32, tag="rb")
                                nc.vector.tensor_copy(rb[:], rb_ps[:])
                                nc.vector.tensor_mul(
                                    OT[qt][po : po + HD, qs], o_ps[0:HD, :], rb[:]
                                )

                    # ---- phase 4: output projection ----
                    with (
                        tc.tile_pool(name="p_wo", bufs=DT + 1) as p_wo,
                        tc.tile_pool(name="p_wraw", bufs=3) as p_wraw,
                        tc.tile_pool(name="p_y", bufs=3) as p_y,
                        tc.tile_pool(name="ps_y", bufs=4, space="PSUM") as ps_y,
                    ):
                        WO = _load_w(nc, p_wo, p_wraw, wo_ext, None)
                        for qc in range(TT):
                            y_t = p_y.tile([P, D], F32, tag="y")
                            for nh in range(2):
                                y_ps = ps_y.tile([P, 512], F32, tag="y")
                                for dt in range(DT + 1):
                                    lhsT = (
                                        OT[dt][:, qc * P : (qc + 1) * P]
                                        if dt < DT
                                        else ones_row[:, qc * P : (qc + 1) * P]
                                    )
                                    nc.tensor.matmul(
                                        y_ps[:],
                                        lhsT,
                                        WO[dt][:, nh * 512 : (nh + 1) * 512],
                                        start=(dt == 0),
                                        stop=(dt == DT),
                                    )
                                nc.any.tensor_copy(y_t[:, nh * 512 : (nh + 1) * 512], y_ps[:])
                            nc.sync.dma_start(out_ext.ap()[qc * P : (qc + 1) * P, :], y_t[:])

    nc.compile()
    return nc


def _transpose_rope_project(tc, nc, x_ext, cs_ext, wqkv_ext, ident, ones_row, all1, s, QT, KT, VA):
    """Build rope'd x^T (pair-split layout, float32r) for set s, then project.

    s=0 (q set): write Q^T tiles into QT.
    s=1 (kv set): write K^T tiles into KT and V (+ones cols) into VA.
    """
    with tc.tile_pool(name=f"p_xt{s}", bufs=DT) as p_xt:
        xT = [p_xt.tile([P, NTOK], F32R, tag="xt", name=f"xt{s}_{i}") for i in range(DT)]

        with (
            tc.tile_pool(name=f"p_cs{s}", bufs=4) as p_cs,
            tc.tile_pool(name=f"p_tmp{s}", bufs=4) as p_tmp,
            tc.tile_pool(name=f"p_raw{s}", bufs=2) as p_raw,
            tc.tile_pool(name=f"ps_t{s}", bufs=4, space="PSUM") as ps_t,
        ):
            # transpose x into pair-split row tiles
            for tt in range(TT):
                raw = p_raw.tile([P, D], F32, tag="raw")
                nc.sync.dma_start(raw[:], x_ext.ap()[tt * P : (tt + 1) * P, :])
                for rt in range(DT):
                    t_ps = ps_t.tile([P, P], F32, tag="t")
                    nc.tensor.transpose(t_ps[:], raw[:, rt * P : (rt + 1) * P], ident[:])
                    nc.any.tensor_copy(xT[rt][:, tt * P : (tt + 1) * P], t_ps[:])

            # rope: groups (even_tile, odd_tile, cos_idx, sin_idx, jtile)
            for g in range(4):
                half = g // 2  # 0: x-half, 1: y-half
                j = g % 2  # jtile
                ev = xT[half * 4 + j]
                od = xT[half * 4 + 2 + j]
                c_t = p_cs.tile([P, NTOK], F32, tag="cs")
                nc.sync.dma_start(c_t[:], cs_ext.ap()[s, 2 * half + 0, j])
                s_t = p_cs.tile([P, NTOK], F32, tag="cs")
                nc.sync.dma_start(s_t[:], cs_ext.ap()[s, 2 * half + 1, j])
                t1 = p_tmp.tile([P, NTOK], F32, tag="tmp")
                t2 = p_tmp.tile([P, NTOK], F32, tag="tmp")
                t3 = p_tmp.tile([P, NTOK], F32, tag="tmp")
                t4 = p_tmp.tile([P, NTOK], F32, tag="tmp")
                nc.vector.tensor_mul(t1[:], ev[:], c_t[:])
                nc.vector.tensor_mul(t2[:], od[:], s_t[:])
                nc.vector.tensor_mul(t3[:], ev[:], s_t[:])
                nc.vector.tensor_mul(t4[:], od[:], c_t[:])
                nc.vector.tensor_sub(ev[:], t1[:], t2[:])
                nc.vector.tensor_add(od[:], t3[:], t4[:])

        # projections
        if s == 0:
            with (
                tc.tile_pool(name="p_wq", bufs=DT + 1) as p_w,
                tc.tile_pool(name="p_wraw0", bufs=3) as p_wraw,
                tc.tile_pool(name="ps_p0", bufs=6, space="PSUM") as ps_p,
            ):
                W = _load_w(nc, p_w, p_wraw, wqkv_ext, 0)
                _proj_T(nc, ps_p, W, xT, ones_row, QT)
        else:
            with (
                tc.tile_pool(name="p_wk", bufs=DT + 1) as p_w,
                tc.tile_pool(name="p_wraw1", bufs=3) as p_wraw,
                tc.tile_pool(name="ps_p1", bufs=6, space="PSUM") as ps_p,
            ):
                W = _load_w(nc, p_w, p_wraw, wqkv_ext, 1)
                _proj_T(nc, ps_p, W, xT, ones_row, KT)
            with (
                tc.tile_pool(name="p_wv", bufs=DT + 1) as p_w,
                tc.tile_pool(name="p_wraw2", bufs=3) as p_wraw,
                tc.tile_pool(name="ps_p2", bufs=6, space="PSUM") as ps_p,
            ):
                W = _load_w(nc, p_w, p_wraw, wqkv_ext, 2)
                # V natural layout with interleaved ones columns
                for tt in range(TT):
                    va = VA[tt]
                    nc.vector.tensor_copy(
                        va[:].rearrange("p (h c) -> p h c", c=HD + 1)[:, :, HD : HD + 1],
                        all1[:].rearrange("p (h c) -> p h c", c=1),
                    )
                    for nh in range(2):
                        v_ps = ps_p.tile([P, 512], F32, tag="p")
                        for dt in range(DT + 1):
                            lhsT = (
                                xT[dt][:, tt * P : (tt + 1) * P]
                                if dt < DT
                                else ones_row[:, tt * P : (tt + 1) * P]
                            )
                            nc.tensor.matmul(
                                v_ps[:],
                                lhsT,
                                W[dt][:, nh * 512 : (nh + 1) * 512],
                                start=(dt == 0),
                                stop=(dt == DT),
                            )
                        out_ap = va[:].rearrange("p (h c) -> p h c", c=HD + 1)[
                            :, nh * 8 : (nh + 1) * 8, 0:HD
                        ]
                        nc.any.tensor_copy(
                            out_ap, v_ps[:].rearrange("p (h c) -> p h c", c=HD)
                        )


def _load_w(nc, pool, rawpool, w_ext, which):
    """DMA one projection's weight tiles and cast-copy them to float32r."""
    W = []
    for dt in range(DT + 1):
        raw = rawpool.tile([P, D], F32, tag="wraw", name=f"wraw{dt}")
        if which is None:
            nc.sync.dma_start(raw[:], w_ext.ap()[dt])
        else:
            nc.sync.dma_start(raw[:], w_ext.ap()[dt, :, which * D : (which + 1) * D])
        w_t = pool.tile([P, D], F32R, tag="w", name=f"w{dt}")
        nc.any.tensor_copy(w_t[:], raw[:])
        W.append(w_t)
    return W


def _proj_T(nc, ps_p, W, xT, ones_row, OUT):
    """OUT[c] = transposed projection [dout-chunk, tok]: lhsT=W slice, rhs=x^T."""
    for c in range(DT):
        out_t = OUT[c]
        for nh in range(2):
            ps = ps_p.tile([P, 512], F32, tag="p")
            for dt in range(DT + 1):
                rhs = xT[dt] if dt < DT else ones_row
                nc.tensor.matmul(
                    ps[:],
                    W[dt][:, c * P : (c + 1) * P],
                    rhs[:, nh * 512 : (nh + 1) * 512],
                    start=(dt == 0),
                    stop=(dt == DT),
                )
            nc.any.tensor_copy(out_t[:, nh * 512 : (nh + 1) * 512], ps[:])


# ---------------------------------------------------------------------------
# host side
# ---------------------------------------------------------------------------

def host_prep(x, coords, is_context, rope_cache,
              ctx_in_w, ctx_in_b, ctx_out_w, ctx_out_b,
              tgt_in_w, tgt_in_b, tgt_out_w, tgt_out_b):
    """Compute per-core input maps + the scatter indices."""
    x = np.asarray(x, np.float32)
    coords = np.asarray(coords, np.float32)
    is_context = np.asarray(is_context, bool)
    rope_cache = np.asarray(rope_cache, np.float32)

    keys = np.where(is_context, 0, 1).astype(np.int32)
    order = np.argsort(keys, axis=1, kind="stable")
    ctx_idx = order[:, :NCTX]
    tgt_idx = order[:, NCTX:]

    # rope positions (mirror reference fp32 arithmetic)
    cn = np.clip(
        coords / np.float32(IMAGE_SIZE) * np.float32(MAX_POS - 1), 0, MAX_POS - 1
    )
    y_pos = cn[..., 0].astype(np.int32)
    x_pos = cn[..., 1].astype(np.int32)
    # [B, K, 256] each
    cx_all = rope_cache[x_pos, :, 0]
    sx_all = rope_cache[x_pos, :, 1]
    cy_all = rope_cache[y_pos, :, 0]
    sy_all = rope_cache[y_pos, :, 1]

    def w_pack(in_w, in_b, out_w, out_b):
        w = np.array(in_w, np.float32)
        bvec = np.array(in_b, np.float32)
        w[0:D] *= np.float32(0.125)
        bvec = bvec.copy()
        bvec[0:D] *= np.float32(0.125)
        wT = np.ascontiguousarray(w.T)[PERM]
        wqkvT = np.concatenate(
            [wT, bvec[None, :], np.zeros((P - 1, 3 * D), np.float32)]
        ).reshape(DT + 1, P, 3 * D)
        woT = np.concatenate(
            [
                np.ascontiguousarray(np.asarray(out_w, np.float32).T),
                np.asarray(out_b, np.float32)[None, :],
                np.zeros((P - 1, D), np.float32),
            ]
        ).reshape(DT + 1, P, D)
        return np.ascontiguousarray(wqkvT), np.ascontiguousarray(woT)

    packs = [w_pack(ctx_in_w, ctx_in_b, ctx_out_w, ctx_out_b),
             w_pack(tgt_in_w, tgt_in_b, tgt_out_w, tgt_out_b)]

    def cs_pack(b, idx):
        # [4, 2, P, NTOK]
        out = np.empty((4, 2, P, NTOK), np.float32)
        for i, arr in enumerate((cx_all, sx_all, cy_all, sy_all)):
            t = arr[b][idx].T  # [256, NTOK]
            out[i] = t.reshape(2, P, NTOK)
        return out

    in_maps = []
    scatter = []
    for c in range(8):
        b, role = c // 2, c % 2
        q_idx = ctx_idx[b] if role == 0 else tgt_idx[b]
        kv_idx = ctx_idx[b]
        ropecs = np.stack([cs_pack(b, q_idx), cs_pack(b, kv_idx)])
        wqkvT, woT = packs[role]
        in_maps.append({
            "xq": np.ascontiguousarray(x[b][q_idx][:, PERM]),
            "xkv": np.ascontiguousarray(x[b][kv_idx][:, PERM]),
            "ropecs": np.ascontiguousarray(ropecs),
            "wqkvT": wqkvT,
            "woT": woT,
        })
        scatter.append((b, q_idx))
    return in_maps, scatter


_NC_CACHE = None


def kernel(**inputs):
    global _NC_CACHE
    in_maps, scatter = host_prep(**inputs)
    if _NC_CACHE is None:
        _NC_CACHE = build_nc()
    nc = _NC_CACHE
    res = run_bass_kernel_spmd(nc, in_maps, core_ids=list(range(8)))
    x = np.asarray(inputs["x"], np.float32)
    out = np.zeros_like(x)
    for c in range(8):
        b, q_idx = scatter[c]
        out[b][q_idx] = res.results[c]["out"]
    return out
